# revision 6
# baseline (speedup 1.0000x reference)
"""LISTA-c (complex LISTA) Trainium2 Bass kernel, 8-core data parallel.

Math (per batch element, complex dim sizes N=128 -> M=256, T=10 iters):
  ys = interleaved real/imag of y            (256-vector)
  Ay = Wa_int @ ys                           (512-vector, interleaved re/im)
  x0 = softshrink_eta(g0 * Ay)
  x_{t} = softshrink_{e_t}(x - g_t*(Wc_int @ x) + g_t*Ay)
        = softshrink(W_t @ x + g_t*Ay),  W_t = I - g_t*Wc_int
  output = x_T de-interleaved to (256, 2)

Layout on chip: features (512, interleaved (m, re/im)) on partitions in 4
chunks of 128; batch on the free dim in tiles of 512. All matmuls in
float32r (TF32-like, ~2^-11 rounding, full PE rate). softshrink(w) =
relu(w - th) - relu(-w - th) with the two relus on ACT (exact: no
cancellation, one side is always zero). g_t*Ay is folded in with a fused
scalar_tensor_tensor on DVE. The final iteration is computed transposed
(batch on partitions) via augmented matmuls so the output DMA is fully
contiguous.
"""

import numpy as np
from contextlib import ExitStack

import concourse.bass as bass
import concourse.bacc as bacc
import concourse.tile as tile
import concourse.mybir as mybir

F32 = mybir.dt.float32
F32R = mybir.dt.float32r
Relu = None  # set lazily
LAMBD = 1.0
NCORES = 8
BATCH = 65536
N = 128          # y complex dim
M = 256          # x complex dim
T = 10
KF = 512         # real feature dim of x (2*M)
KY = 256         # real feature dim of y (2*N)
FT = 512         # batch tile (free dim)
PER_CORE = BATCH // NCORES
NTILES = PER_CORE // FT


def _interleave_cw(W0, W1):
    """Complex matrix (W0 + i W1), (m, n) -> real interleaved (2m, 2n):
    out[2a+c, 2b+d] so that out @ interleave(x) = interleave(W x)."""
    m, n = W0.shape
    W = np.zeros((2 * m, 2 * n), dtype=np.float64)
    W[0::2, 0::2] = W0
    W[0::2, 1::2] = -W1
    W[1::2, 0::2] = W1
    W[1::2, 1::2] = W0
    return W


def build_nc(etas, gammas):
    """etas/gammas: python floats list of length T+1 (baked as immediates)."""
    nc = bacc.Bacc("TRN2", target_bir_lowering=False, debug=False,
                   num_devices=NCORES)
    COPY = mybir.ActivationFunctionType.Copy
    RELU = mybir.ActivationFunctionType.Relu
    ALU = mybir.AluOpType

    # weight pack layout (columns):
    #  wts:  (T-1)*4*4*128   [t=1..9][kc][j]  lhsT chunks of W_t
    #  wat:  2*4*128         [kc][j]          lhsT chunks of Wa_int
    #  w10:  4*512           [kc]             rhs wide chunks of W_10^T
    #  giw:  4*512           [kc]             rhs wide chunks of g10*I
    #  id:   128
    n_wts = (T - 1) * 16 * 128
    n_wat = 8 * 128
    n_w10 = 4 * 512
    n_giw = 4 * 512
    WPK_COLS = n_wts + n_wat + n_w10 + n_giw + 128
    o_wat = n_wts
    o_w10 = o_wat + n_wat
    o_giw = o_w10 + n_w10
    o_id = o_giw + n_giw

    yv = nc.declare_dram_parameter("yv", [PER_CORE, KY], F32R, isOutput=False)
    wpk = nc.declare_dram_parameter("wpk", [128, WPK_COLS], F32R, isOutput=False)
    out = nc.declare_dram_parameter("out", [PER_CORE, KF], F32, isOutput=True)

    th = [float(e) * LAMBD for e in etas]
    g = [float(x) for x in gammas]

    with tile.TileContext(nc) as tc, ExitStack() as ctx:
        wp = ctx.enter_context(tc.tile_pool(name="wp", bufs=1))
        ysbp = ctx.enter_context(tc.tile_pool(name="ysbp", bufs=1))
        ysp = ctx.enter_context(tc.tile_pool(name="ysp", bufs=2))
        ayp = ctx.enter_context(tc.tile_pool(name="ayp", bufs=2))
        xp = ctx.enter_context(tc.tile_pool(name="xp", bufs=4))
        wwp = ctx.enter_context(tc.tile_pool(name="wwp", bufs=2))
        pnp = ctx.enter_context(tc.tile_pool(name="pnp", bufs=3))
        osbp = ctx.enter_context(tc.tile_pool(name="osbp", bufs=2))
        psmm = ctx.enter_context(tc.tile_pool(name="psmm", bufs=1, space="PSUM"))
        psyt = ctx.enter_context(tc.tile_pool(name="psyt", bufs=1, space="PSUM"))

        wt = wp.tile([128, WPK_COLS], F32R)
        nc.sync.dma_start(wt[:], wpk[:])

        # per-threshold bias const tiles (activation bias must be an SBUF AP)
        biasp = ctx.enter_context(tc.tile_pool(name="biasp", bufs=1))
        bias_by_val = {}
        bias_t = []
        for t in range(T + 1):
            v = -th[t]
            if v not in bias_by_val:
                bt = biasp.tile([128, 1], F32, tag=f"bias{len(bias_by_val)}")
                nc.vector.memset(bt[:], v)
                bias_by_val[v] = bt
            bias_t.append(bias_by_val[v])

        def wts_ap(t, kc, j):  # lhsT (128k, 128m) of W_t chunk
            off = (((t - 1) * 4 + kc) * 4 + j) * 128
            return wt[:, off:off + 128]

        def wat_ap(kc, j):
            off = o_wat + (kc * 4 + j) * 128
            return wt[:, off:off + 128]

        def w10_ap(kc):
            off = o_w10 + kc * 512
            return wt[:, off:off + 512]

        def giw_ap(kc):
            off = o_giw + kc * 512
            return wt[:, off:off + 512]

        ident = wt[:, o_id:o_id + 128]

        def prologue(ti, tag):
            b0 = ti * FT
            ysb = ysbp.tile([128, 4, KY], F32R, tag="ysb")
            src = yv[b0:b0 + FT, :].rearrange("(s p) f -> p s f", p=128)
            nc.sync.dma_start(ysb[:], src)
            # transpose y into feature-major ys (2 chunks of 128 feats)
            ys = ysp.tile([128, 2, FT], F32R, tag="ys")
            for h in range(2):
                ytp = psyt.tile([128, FT], F32R, tag=f"ytr{h}")
                for s in range(4):
                    nc.tensor.transpose(ytp[:, s * 128:(s + 1) * 128],
                                        ysb[:, s, h * 128:(h + 1) * 128],
                                        ident)
                nc.scalar.activation(ys[:, h, :], ytp[:], COPY)
            # Ay = Wa_int @ ys ; x0 = softshrink(g0*Ay, th0)
            ay = ayp.tile([128, 4, FT], F32R, tag="ay")
            x = xp.tile([128, 4, FT], F32R, tag=f"x{tag}")
            for j in range(4):
                psa = psmm.tile([128, FT], F32, tag=f"{tag}{j % 2}")
                for kc in range(2):
                    nc.tensor.matmul(psa[:], wat_ap(kc, j), ys[:, kc, :],
                                     start=(kc == 0), stop=(kc == 1))
                nc.vector.tensor_copy(ay[:, j, :], psa[:])
                p = pnp.tile([128, FT], F32, tag="p")
                n = pnp.tile([128, FT], F32, tag="n")
                nc.scalar.activation(p[:], psa[:], RELU, bias=bias_t[0][:], scale=g[0])
                nc.scalar.activation(n[:], psa[:], RELU, bias=bias_t[0][:], scale=-g[0])
                eng = nc.vector if j % 2 == 0 else nc.gpsimd
                eng.tensor_tensor(x[:, j, :], p[:], n[:], ALU.subtract)
            return ay, x

        def iter_mms(t, tag, x, j):
            ps = psmm.tile([128, FT], F32, tag=f"{tag}{j % 2}")
            for k in range(4):
                nc.tensor.matmul(ps[:], wts_ap(t, k, j), x[:, k, :],
                                 start=(k == 0), stop=(k == 3))
            return ps

        def iter_tail(t, tag, ay, ps, xn, j):
            w = wwp.tile([128, FT], F32, tag="w")
            nc.vector.scalar_tensor_tensor(w[:], ay[:, j, :], g[t], ps[:],
                                           ALU.mult, ALU.add)
            p = pnp.tile([128, FT], F32, tag="p")
            n = pnp.tile([128, FT], F32, tag="n")
            nc.scalar.activation(p[:], w[:], RELU, bias=bias_t[t][:], scale=1.0)
            nc.scalar.activation(n[:], w[:], RELU, bias=bias_t[t][:], scale=-1.0)
            eng = nc.vector if j % 2 == 0 else nc.gpsimd
            eng.tensor_tensor(xn[:, j, :], p[:], n[:], ALU.subtract)

        def last_iter(ti, tag, ay, x):
            b0 = ti * FT
            for s in range(4):
                ps = psmm.tile([128, FT], F32, tag=f"{tag}{s % 2}")
                sl = slice(s * 128, (s + 1) * 128)
                for k in range(4):
                    nc.tensor.matmul(ps[:], x[:, k, sl], w10_ap(k),
                                     start=(k == 0), stop=False)
                for k in range(4):
                    nc.tensor.matmul(ps[:], ay[:, k, sl], giw_ap(k),
                                     start=False, stop=(k == 3))
                p = pnp.tile([128, FT], F32, tag="p")
                n = pnp.tile([128, FT], F32, tag="n")
                nc.scalar.activation(p[:], ps[:], RELU, bias=bias_t[T][:], scale=1.0)
                nc.scalar.activation(n[:], ps[:], RELU, bias=bias_t[T][:], scale=-1.0)
                osb = osbp.tile([128, FT], F32, tag="osb")
                eng = nc.vector if s % 2 == 0 else nc.gpsimd
                eng.tensor_tensor(osb[:], p[:], n[:], ALU.subtract)
                nc.sync.dma_start(out[b0 + s * 128:b0 + (s + 1) * 128, :], osb[:])

        for pair in range(NTILES // 2):
            tiles = (2 * pair, 2 * pair + 1)
            st = {}
            for ti, tag in zip(tiles, ("A", "B")):
                ay, x = prologue(ti, tag)
                st[tag] = [ay, x]
            for t in range(1, T):
                newx = {}
                for tag in ("A", "B"):
                    newx[tag] = xp.tile([128, 4, FT], F32R, tag=f"x{tag}", name=f"x{tag}_{t}")
                for j in range(4):
                    pss = {}
                    for tag in ("A", "B"):
                        pss[tag] = iter_mms(t, tag, st[tag][1], j)
                    for tag in ("A", "B"):
                        iter_tail(t, tag, st[tag][0], pss[tag], newx[tag], j)
                for tag in ("A", "B"):
                    st[tag][1] = newx[tag]
            for ti, tag in zip(tiles, ("A", "B")):
                last_iter(ti, tag, st[tag][0], st[tag][1])

    nc.compile()
    return nc


def host_pack(A, B, etas, gammas):
    """Build the packed weight tensor (128, WPK_COLS) float32."""
    g = [float(x) for x in np.asarray(gammas).reshape(-1)]
    Wa = _interleave_cw(A[0].astype(np.float64), A[1].astype(np.float64))   # (512, 256)
    Wc = _interleave_cw(B[0].astype(np.float64), B[1].astype(np.float64))   # (512, 512)
    I = np.eye(KF)

    cols = []
    # wts: t=1..9, lhsT[kk, mm] = W_t[j*128+mm, kc*128+kk]
    for t in range(1, T):
        Wt = I - g[t] * Wc
        for kc in range(4):
            for j in range(4):
                blk = Wt[j * 128:(j + 1) * 128, kc * 128:(kc + 1) * 128].T
                cols.append(blk)
    # wat: lhsT[kk, mm] = Wa[j*128+mm, kc*128+kk]
    for kc in range(2):
        for j in range(4):
            blk = Wa[j * 128:(j + 1) * 128, kc * 128:(kc + 1) * 128].T
            cols.append(blk)
    # w10: rhs wide [kk, m] = W_10[m, kc*128+kk]
    W10 = I - g[T] * Wc
    for kc in range(4):
        cols.append(W10[:, kc * 128:(kc + 1) * 128].T)   # (128, 512)
    # giw: [kk, m] = g10 * (m == kc*128+kk)
    for kc in range(4):
        blk = np.zeros((128, KF))
        for kk in range(128):
            blk[kk, kc * 128 + kk] = g[T]
        cols.append(blk)
    cols.append(np.eye(128))
    return np.concatenate(cols, axis=1).astype(np.float32)


def _run(nc, in_maps):
    from concourse import bass2jax
    return bass2jax.run_bass_via_pjrt(nc, in_maps, n_cores=NCORES)


def kernel(y, A, B, etas, gammas):
    y = np.ascontiguousarray(np.asarray(y, dtype=np.float32))
    A = np.asarray(A, dtype=np.float32)
    B = np.asarray(B, dtype=np.float32)
    ev = [float(x) for x in np.asarray(etas, dtype=np.float32).reshape(-1)]
    gv = [float(x) for x in np.asarray(gammas, dtype=np.float32).reshape(-1)]

    nc = build_nc(ev, gv)
    wpk = host_pack(A, B, ev, gv)
    yflat = y.reshape(BATCH, KY)
    in_maps = [{"yv": yflat[c * PER_CORE:(c + 1) * PER_CORE], "wpk": wpk}
               for c in range(NCORES)]
    res = _run(nc, in_maps)
    outs = [res[c]["out"] for c in range(NCORES)]
    full = np.concatenate(outs, axis=0)          # (BATCH, 512)
    return full.reshape(BATCH, M, 2)
